# revision 1
# baseline (speedup 1.0000x reference)
"""Trainium2 Bass kernel for nn_BCMEmulator (TCN emulator).

Model: 5-block dilated-causal-conv TCN (CH=64, K=3, dils 1,2,4,8,16) over
(B=128, T=1024) + pointwise heads (pet/pck softplus, aet sigmoid gate, cwd).

Strategy (pure data parallel, 8 cores x 16 sequences):
 - Each core processes 16 sequences as 8 "pairs". A pair packs 2 sequences
   into the 128 SBUF partitions: rows 0-63 = seq A channels, 64-127 = seq B.
 - Every conv tap is one matmul (K=128 contraction = 2x64 channels,
   block-diagonal weights, M=128 = 2x64 output channels, N=512 time cols).
   Causal dilation is a column offset into a left-zero-padded SBUF tensor.
 - float32r matmuls: 1 PE cycle/row for N>=256 (bf16 speed), ~1.3e-4 rel err.
 - ReLU on ScalarE (free per-partition bias), residual add fused on VectorE
   via scalar_tensor_tensor (f = max(psB,0) + f) when biases are zero.
 - softplus = ln(1+exp(.)), sigmoid(z) = exp(-ln(1+exp(-z))): only the
   natural_log_exp_and_others ACT table set is used (no table switches).
"""
import sys

sys.path.insert(0, "/opt/trn_rl_repo")

import numpy as np

import concourse.bacc as bacc
import concourse.bass as bass
import concourse.tile as tile
from concourse import mybir
from concourse.bass_utils import run_bass_kernel_spmd

B, T = 128, 1024
C_IN, EMB = 15, 8
CH = 64
DILS = [1, 2, 4, 8, 16]
CT = C_IN + EMB              # 23 input channels after fveg concat
NCORES = 8
BPC = B // NCORES            # 16 sequences per core
NPAIR = BPC // 2             # 8 pairs per core
P0 = 2 * DILS[-1]            # 32 left-pad columns (max lookback)
PADT = P0 + T
TT = 512                     # matmul free-dim tile (one PSUM bank of fp32)
NTT = T // TT

F32R = mybir.dt.float32r
F32 = mybir.dt.float32
AF = mybir.ActivationFunctionType
ALU = mybir.AluOpType

_PROGRAM_CACHE = {}


def _pin_act_table():
    """Force every ACT instruction onto natural_log_exp_and_others (which
    contains Relu+Exp+Ln): the greedy per-instruction set picker otherwise
    thrashes Relu/Exp->set0, Ln->set5, inserting ~33 table loads (~2.7us
    each, serializing ScalarE). Membership is edited, order preserved, so
    emitted act_func_set_ids still index act_info.json correctly."""
    import concourse.hw_specs as hw_specs
    if getattr(bacc.get_activation_tables, "_pinned", False):
        return
    orig = bacc.get_activation_tables
    mine = {AF.Relu, AF.Exp, AF.Ln}

    def patched(arch):
        tabs = orig(arch)
        return {
            name: (set(fns) if name == "natural_log_exp_and_others"
                   else set(fns) - mine)
            for name, fns in tabs.items()
        }

    patched._pinned = True
    bacc.get_activation_tables = patched
    hw_specs_patched = patched
    del hw_specs_patched


def build_program(zero_bb):
    _pin_act_table()
    """Build + compile the per-core Bass program.

    zero_bb: tuple of 4 bools — whether bb[i] is all-zero (enables the fused
    DVE relu+residual-add; otherwise an extra ACT relu-with-bias is emitted).
    """
    nc = bacc.Bacc("TRN2", target_bir_lowering=False, debug=False,
                   num_devices=NCORES)

    xin_d = nc.dram_tensor("xin", [NPAIR, 2 * CT, 1 + T], F32R, kind="ExternalInput")
    w0_d = nc.dram_tensor("w0", [4 * CT, 3, 128], F32R, kind="ExternalInput")
    wk_d = nc.dram_tensor("wk", [64, 27, 64], F32R, kind="ExternalInput")
    whp_d = nc.dram_tensor("whp", [128, 4], F32R, kind="ExternalInput")
    wha_d = nc.dram_tensor("wha", [128, 2], F32R, kind="ExternalInput")
    wh2_d = nc.dram_tensor("wh2", [4, 2], F32R, kind="ExternalInput")
    bias_d = nc.dram_tensor("bias", [128, 11], F32, kind="ExternalInput")
    bh_d = nc.dram_tensor("bh", [4, 2], F32, kind="ExternalInput")
    out_d = {
        nm: nc.dram_tensor(nm, [BPC, T], F32, kind="ExternalOutput")
        for nm in ("pet", "pck", "aet", "cwd")
    }

    with tile.TileContext(nc) as tc:
        with (
            tc.tile_pool(name="wpool", bufs=1) as wpool,
            tc.tile_pool(name="xpool", bufs=3) as xpool,
            tc.tile_pool(name="fpool", bufs=6) as fpool,
            tc.tile_pool(name="hpool", bufs=6) as hpool,
            tc.tile_pool(name="spool", bufs=2) as spool,
            tc.tile_pool(name="pspool", bufs=1, space=bass.MemorySpace.PSUM) as ps,
        ):
            w0_sb = wpool.tile([4 * CT, 3, 128], F32R)
            wk_sb = wpool.tile([128, 27, 128], F32R)
            whp_sb = wpool.tile([128, 4], F32R)
            wha_sb = wpool.tile([128, 2], F32R)
            wh2_sb = wpool.tile([4, 2], F32R)
            bias_sb = wpool.tile([128, 11], F32)
            bh_sb = wpool.tile([4, 2], F32)
            nc.gpsimd.dma_start(out=w0_sb, in_=w0_d[:])
            # wk ships as (64,27,64); the block-diagonal (128,27,128) lhsT is
            # assembled on-device: zero the tile once, then 2 DMAs fill the
            # diagonal quadrants (weights are static, so this runs once)
            nc.vector.memset(wk_sb.bitcast(F32), 0.0)

            # preload all pair inputs; the big wk DMA is issued after the
            # first pairs' xin so pair 0's conv0a isn't queued behind it
            xins = []
            for p in range(NPAIR):
                xin_sb = xpool.tile([4 * CT, T], F32R, tag=f"xin{p}",
                                    name=f"xin_sb{p}", bufs=1)
                # rows 0-45: xin; rows 46-91: xin right-shifted by one column
                # (the dram copy has one leading zero column, so the shifted
                # view is just the same dram region starting one col earlier)
                nc.sync.dma_start(out=xin_sb[0:2 * CT, :],
                                  in_=xin_d[p, :, 1:1 + T])
                eng = nc.gpsimd if p < 2 else nc.sync
                eng.dma_start(out=xin_sb[2 * CT:4 * CT, :],
                              in_=xin_d[p, :, 0:T])
                xins.append(xin_sb)
                if p == 0:
                    nc.sync.dma_start(out=bias_sb, in_=bias_d[:])
                if p == 1:
                    nc.sync.dma_start(out=bh_sb, in_=bh_d[:])
                    nc.sync.dma_start(out=wk_sb[0:64, :, 0:64], in_=wk_d[:])
                    nc.sync.dma_start(out=wk_sb[64:128, :, 64:128], in_=wk_d[:])
                    nc.sync.dma_start(out=whp_sb, in_=whp_d[:])
                    nc.sync.dma_start(out=wha_sb, in_=wha_d[:])
                    nc.sync.dma_start(out=wh2_sb, in_=wh2_d[:])

            def conv_taps(psum, lhsT_of_j, src, d, base_k):
                """3-tap dilated causal conv: psum += sum_j W_j @ src shifted
                right by s=(2-j)*d. Causal zero-padding falls out of PSUM
                has_written semantics: tap0 (start=True) clears the bank and
                writes only cols [s0:TT]; later taps overwrite the still-
                unwritten left edge and accumulate elsewhere."""
                for j in (2, 1, 0):
                    s = (2 - j) * d
                    for t in range(NTT):
                        lo = t * TT
                        out_lo = lo + (s if t == 0 else 0)
                        nc.tensor.matmul(
                            psum[:, out_lo:lo + TT],
                            lhsT_of_j(j),
                            src[:base_k, out_lo - s:lo + TT - s],
                            start=(j == 2),
                            stop=(j == 0),
                        )

            GRP = 2

            def conv_taps(psum, lhsT_of_j, src, d, base_k=128):
                """3-tap dilated causal conv: psum += sum_j W_j @ src shifted
                right by s=(2-j)*d. Causal zero-padding falls out of PSUM
                has_written semantics: the shift-0 tap goes first (start=True,
                full width, clears the bank); shifted taps then accumulate
                into fully-written regions, leaving the left edge untouched
                where their input would be out of range."""
                for j in (2, 1, 0):
                    s = (2 - j) * d
                    for t in range(NTT):
                        lo = t * TT
                        out_lo = lo + (s if t == 0 else 0)
                        nc.tensor.matmul(
                            psum[:, out_lo:lo + TT],
                            lhsT_of_j(j),
                            src[:base_k, out_lo - s:lo + TT - s],
                            start=(j == 2),
                            stop=(j == 0),
                        )

            # Two-level software pipeline, stage-interleaved emission:
            #  - pairs are processed in groups of GRP=2; within each stage the
            #    per-pair ops are emitted round-robin so every engine has
            #    same-stage work from both pairs queued;
            #  - the previous group's head stages are drained between the
            #    current group's block phases, so head transcendentals overlap
            #    the next group's conv matmuls and only the last group's heads
            #    form the kernel tail.
            # PSUM: tag p%4 (one (128,1024) = 2-bank slot per pair), so
            # consecutive groups use disjoint tag pairs {0,1}/{2,3}.
            st = {}

            def blk0_convA(grp):
                for p in grp:
                    psA = ps.tile([128, T], F32, tag=f"ps{p % 4}",
                                  name=f"psA0_{p}")
                    for t in range(NTT):
                        lo = t * TT
                        nc.tensor.matmul(psA[:, lo:lo + TT], w0_sb[0:92, 0, :],
                                         xins[p][0:92, lo:lo + TT],
                                         start=True, stop=False)
                    for t in range(NTT):
                        lo = t * TT
                        out_lo = lo + (2 if t == 0 else 0)
                        nc.tensor.matmul(psA[:, out_lo:lo + TT],
                                         w0_sb[0:46, 1, :],
                                         xins[p][0:46, out_lo - 2:lo + TT - 2],
                                         start=False, stop=True)
                    st[p] = {"ps": psA}
                for p in grp:
                    h1 = hpool.tile([128, T], F32R, tag="h1", name=f"h1b0_{p}")
                    nc.scalar.activation(out=h1, in_=st[p]["ps"], func=AF.Relu,
                                         bias=bias_sb[:, 0:1], scale=1.0)
                    st[p]["h1"] = h1
                    h1s = hpool.tile([128, T], F32R, tag="h1s", name=f"h1s_{p}")
                    nc.vector.tensor_scalar(out=h1s[:, 1:T],
                                            in0=st[p]["ps"][:, 0:T - 1],
                                            scalar1=bias_sb[:, 0:1],
                                            scalar2=0.0,
                                            op0=ALU.add, op1=ALU.max)
                    nc.vector.tensor_scalar_mul(out=h1s[:, 0:1],
                                                in0=bias_sb[:, 0:1],
                                                scalar1=0.0)
                    st[p]["h1s"] = h1s

            def blk0_convB(grp):
                for p in grp:
                    psB = ps.tile([128, T], F32, tag=f"ps{p % 4}",
                                  name=f"psB0_{p}")
                    h1, h1s = st[p]["h1"], st[p]["h1s"]
                    for t in range(NTT):
                        lo = t * TT
                        nc.tensor.matmul(psB[:, lo:lo + TT], wk_sb[:, 2, :],
                                         h1[:, lo:lo + TT],
                                         start=True, stop=False)
                    for t in range(NTT):
                        lo = t * TT
                        nc.tensor.matmul(psB[:, lo:lo + TT], wk_sb[:, 1, :],
                                         h1s[:, lo:lo + TT],
                                         start=False, stop=False)
                    for t in range(NTT):
                        lo = t * TT
                        out_lo = lo + (2 if t == 0 else 0)
                        nc.tensor.matmul(psB[:, out_lo:lo + TT], wk_sb[:, 0, :],
                                         h1[:, out_lo - 2:lo + TT - 2],
                                         start=False, stop=True)
                    st[p]["ps"] = psB
                for p in grp:
                    h2 = hpool.tile([128, T], F32, tag="h2", name=f"h2_{p}",
                                    bufs=5)
                    nc.vector.tensor_scalar(out=h2, in0=st[p]["ps"],
                                            scalar1=bias_sb[:, 1:2],
                                            scalar2=0.0,
                                            op0=ALU.add, op1=ALU.max)
                    st[p]["h2"] = h2

            def blk0_resid(grp):
                for p in grp:
                    psR = ps.tile([128, T], F32, tag=f"ps{p % 4}",
                                  name=f"psR_{p}")
                    for t in range(NTT):
                        nc.tensor.matmul(
                            psR[:, t * TT:(t + 1) * TT], w0_sb[0:46, 2, :],
                            xins[p][0:46, t * TT:(t + 1) * TT],
                            start=True, stop=True)
                    st[p]["ps"] = psR
                for p in grp:
                    f = fpool.tile([128, T], F32R, tag="f", name=f"f_{p}")
                    nc.vector.scalar_tensor_tensor(
                        out=f, in0=st[p]["ps"], scalar=bias_sb[:, 2:3],
                        in1=st[p]["h2"], op0=ALU.add, op1=ALU.add)
                    st[p]["f"] = f

            def blk(grp, i, d):
                for p in grp:
                    psA = ps.tile([128, T], F32, tag=f"ps{p % 4}",
                                  name=f"psA{i + 1}_{p}")
                    conv_taps(psA, lambda j: wk_sb[:, 3 + 6 * i + j, :],
                              st[p]["f"], d)
                    st[p]["ps"] = psA
                for p in grp:
                    h1 = hpool.tile([128, T], F32R, tag="h1",
                                    name=f"h1_{i + 1}_{p}")
                    nc.scalar.activation(out=h1, in_=st[p]["ps"], func=AF.Relu,
                                         bias=bias_sb[:, 3 + i:4 + i],
                                         scale=1.0)
                    st[p]["h1"] = h1
                for p in grp:
                    psB = ps.tile([128, T], F32, tag=f"ps{p % 4}",
                                  name=f"psB{i + 1}_{p}")
                    conv_taps(psB, lambda j: wk_sb[:, 6 + 6 * i + j, :],
                              st[p]["h1"], d)
                    st[p]["ps"] = psB
                for p in grp:
                    if zero_bb[i]:
                        nc.vector.scalar_tensor_tensor(
                            out=st[p]["f"], in0=st[p]["ps"], scalar=0.0,
                            in1=st[p]["f"].bitcast(F32),
                            op0=ALU.max, op1=ALU.add)
                    else:
                        h2 = hpool.tile([128, T], F32, tag="h2",
                                        name=f"h2_{i + 1}_{p}", bufs=5)
                        nc.scalar.activation(out=h2, in_=st[p]["ps"],
                                             func=AF.Relu,
                                             bias=bias_sb[:, 7 + i:8 + i],
                                             scale=1.0)
                        nc.vector.tensor_tensor(
                            out=st[p]["f"], in0=st[p]["f"].bitcast(F32),
                            in1=h2, op=ALU.add)

            def head_stages(grp):
                def s_mmH():
                    for p in grp:
                        psH = ps.tile([4, T], F32, tag=f"ps{p % 4}",
                                      name=f"psH_{p}")
                        for t in range(NTT):
                            sl = slice(t * TT, (t + 1) * TT)
                            nc.tensor.matmul(psH[:, sl], whp_sb,
                                             st[p]["f"][:, sl],
                                             start=True, stop=True)
                        st[p]["ps"] = psH

                def s_spe():
                    for p in grp:
                        spe = spool.tile([4, T], F32, tag="spe",
                                         name=f"spe_{p}")
                        nc.scalar.activation(out=spe, in_=st[p]["ps"],
                                             func=AF.Exp,
                                             bias=bh_sb[:, 0:1], scale=1.0)
                        st[p]["spe"] = spe

                def s_sp():
                    for p in grp:
                        sp = spool.tile([4, T], F32R, tag="sp",
                                        name=f"sp_{p}", bufs=5)
                        nc.scalar.activation(out=sp, in_=st[p]["spe"],
                                             func=AF.Ln, bias=1.0, scale=1.0)
                        st[p]["sp"] = sp

                def s_mmH2():
                    for p in grp:
                        psH2 = ps.tile([2, T], F32, tag=f"ps{p % 4}",
                                       name=f"psH2_{p}")
                        for t in range(NTT):
                            sl = slice(t * TT, (t + 1) * TT)
                            nc.tensor.matmul(psH2[:, sl], wha_sb,
                                             st[p]["f"][:, sl],
                                             start=True, stop=False)
                        for t in range(NTT):
                            sl = slice(t * TT, (t + 1) * TT)
                            nc.tensor.matmul(psH2[:, sl], wh2_sb,
                                             st[p]["sp"][:, sl],
                                             start=False, stop=True)
                        st[p]["ps"] = psH2

                def s_ge():
                    for p in grp:
                        ge = spool.tile([2, T], F32, tag="ge", name=f"ge_{p}")
                        nc.scalar.activation(out=ge, in_=st[p]["ps"],
                                             func=AF.Exp,
                                             bias=bh_sb[0:2, 1:2], scale=-1.0)
                        st[p]["ge"] = ge

                def s_gl():
                    for p in grp:
                        gl = spool.tile([2, T], F32, tag="gl", name=f"gl_{p}")
                        nc.scalar.activation(out=gl, in_=st[p]["ge"],
                                             func=AF.Ln, bias=1.0, scale=1.0)
                        st[p]["gl"] = gl

                def s_gg():
                    for p in grp:
                        gg = spool.tile([2, T], F32, tag="gg", name=f"gg_{p}")
                        nc.scalar.activation(out=gg, in_=st[p]["gl"],
                                             func=AF.Exp, scale=-1.0)
                        st[p]["gg"] = gg

                def s_petdma():
                    for p in grp:
                        nc.sync.dma_start(
                            out=out_d["pet"][2 * p:2 * p + 2, :],
                            in_=st[p]["sp"].bitcast(F32)[0:2, :])
                        nc.sync.dma_start(
                            out=out_d["pck"][2 * p:2 * p + 2, :],
                            in_=st[p]["sp"].bitcast(F32)[2:4, :])

                def s_aet():
                    for p in grp:
                        aet = spool.tile([2, T], F32, tag="aet",
                                         name=f"aet_{p}")
                        nc.vector.tensor_tensor(
                            out=aet, in0=st[p]["gg"],
                            in1=st[p]["sp"].bitcast(F32)[0:2, :], op=ALU.mult)
                        st[p]["aet"] = aet

                def s_cwd():
                    for p in grp:
                        cwd = spool.tile([2, T], F32, tag="cwd",
                                         name=f"cwd_{p}")
                        nc.vector.tensor_tensor(
                            out=cwd, in0=st[p]["sp"].bitcast(F32)[0:2, :],
                            in1=st[p]["aet"], op=ALU.subtract)
                        st[p]["cwd"] = cwd

                def s_aetdma():
                    for p in grp:
                        nc.sync.dma_start(
                            out=out_d["aet"][2 * p:2 * p + 2, :],
                            in_=st[p]["aet"])

                def s_cwddma():
                    for p in grp:
                        nc.sync.dma_start(
                            out=out_d["cwd"][2 * p:2 * p + 2, :],
                            in_=st[p]["cwd"])

                return [s_mmH, s_spe, s_sp, s_petdma, s_mmH2, s_ge, s_gl,
                        s_gg, s_aet, s_aetdma, s_cwd, s_cwddma]

            pending = []

            def drain(n):
                for _ in range(n):
                    if pending:
                        pending.pop(0)()

            for g0 in range(0, NPAIR, GRP):
                grp = list(range(g0, min(g0 + GRP, NPAIR)))
                phases = [lambda: blk0_convA(grp), lambda: blk0_convB(grp),
                          lambda: blk0_resid(grp)]
                for i, d in enumerate(DILS[1:]):
                    phases.append(lambda i=i, d=d: blk(grp, i, d))
                for ph in phases:
                    ph()
                    drain(2)
                drain(len(pending))
                pending = head_stages(grp)
            for s in pending:
                s()

    nc.compile()
    return nc


def get_program(zero_bb):
    key = tuple(zero_bb)
    if key not in _PROGRAM_CACHE:
        _PROGRAM_CACHE[key] = build_program(key)
    return _PROGRAM_CACHE[key]


def prep_inputs(inputs):
    """Host-side packing: returns (zero_bb, shared weight map, per-core xin)."""
    g = {k: np.asarray(v) for k, v in inputs.items()}
    x = g["x"].astype(np.float32, copy=False)
    ids = g["fveg_ids"].astype(np.int64)
    emb = g["fveg_emb"].astype(np.float32, copy=False)

    fv = emb[ids]                                     # (B, EMB)
    xin = np.concatenate(
        [x, np.broadcast_to(fv[:, :, None], (B, EMB, T))], axis=1)  # (B,23,T)
    xin_pad = np.zeros((B, CT, 1 + T), np.float32)
    xin_pad[:, :, 1:] = xin
    xin_cores = np.ascontiguousarray(
        xin_pad.reshape(NCORES, NPAIR, 2 * CT, 1 + T))

    w0 = np.zeros((4 * CT, 3, 128), np.float32)
    w0a, w0r = g["w0a"].astype(np.float32), g["w0r"].astype(np.float32)
    for s in range(2):                  # seq-in-pair
        r0, c0 = s * CT, s * 64
        w0[r0:r0 + CT, 0, c0:c0 + 64] = w0a[:, :, 2].T          # tap2, x
        w0[46 + r0:46 + r0 + CT, 0, c0:c0 + 64] = w0a[:, :, 1].T  # tap1, xsh
        w0[r0:r0 + CT, 1, c0:c0 + 64] = w0a[:, :, 0].T          # tap0
        w0[r0:r0 + CT, 2, c0:c0 + 64] = w0r[:, :, 0].T          # 1x1 resid
    

    wk = np.zeros((27, 64, 64), np.float32)
    for j in range(3):
        wk[j] = g["w0b"].astype(np.float32)[:, :, j].T
    wa, wb = g["wa"].astype(np.float32), g["wb"].astype(np.float32)
    for i in range(4):
        for j in range(3):
            wk[3 + 6 * i + j] = wa[i, :, :, j].T
            wk[6 + 6 * i + j] = wb[i, :, :, j].T
    wk = np.ascontiguousarray(wk.transpose(1, 0, 2))  # (64, 27, 64)

    pet_w = g["pet_w"].astype(np.float32)[0, :, 0]    # (64,)
    pck_w = g["pck_w"].astype(np.float32)[0, :, 0]
    aet_w = g["aet_w"].astype(np.float32)[0, :, 0]    # (66,)
    whp = np.zeros((128, 4), np.float32)
    whp[0:64, 0] = pet_w
    whp[64:128, 1] = pet_w
    whp[0:64, 2] = pck_w
    whp[64:128, 3] = pck_w
    wha = np.zeros((128, 2), np.float32)
    wha[0:64, 0] = aet_w[0:64]
    wha[64:128, 1] = aet_w[0:64]
    wpet, wpck = aet_w[64], aet_w[65]
    wh2 = np.array([[wpet, 0], [0, wpet], [wpck, 0], [0, wpck]], np.float32)

    bcols = [g["b0a"], g["b0b"], g["b0r"]] + [g["ba"][i] for i in range(4)] \
        + [g["bb"][i] for i in range(4)]
    bias = np.stack([np.tile(c.astype(np.float32), 2) for c in bcols], axis=1)

    pet_b = float(g["pet_b"][0])
    pck_b = float(g["pck_b"][0])
    aet_b = float(g["aet_b"][0])
    bh = np.array([[pet_b, -aet_b], [pet_b, -aet_b],
                   [pck_b, 0.0], [pck_b, 0.0]], np.float32)

    zero_bb = tuple(bool(np.all(g["bb"][i] == 0)) for i in range(4))
    shared = {"w0": w0, "wk": wk, "whp": whp, "wha": wha, "wh2": wh2,
              "bias": bias, "bh": bh}
    return zero_bb, shared, xin_cores


def run(inputs, trace=False, trace_kwargs=None):
    zero_bb, shared, xin_cores = prep_inputs(inputs)
    nc = get_program(zero_bb)
    in_maps = [
        {"xin": np.ascontiguousarray(xin_cores[c]), **shared}
        for c in range(NCORES)
    ]
    res = run_bass_kernel_spmd(nc, in_maps, core_ids=list(range(NCORES)),
                               trace=trace, **(trace_kwargs or {}))
    outs = []
    for nm in ("pet", "pck", "aet", "cwd"):
        full = np.concatenate([res.results[c][nm] for c in range(NCORES)], 0)
        outs.append(full.reshape(B, 1, T).astype(np.float32))
    return tuple(outs), res


def kernel(**inputs):
    outs, _ = run(inputs)
    return outs


def build_calib():
    """Same I/O signature as the real program, minimal compute — used by the
    bench to measure the axon relay's per-exec input-staging overhead."""
    _pin_act_table()
    nc = bacc.Bacc("TRN2", target_bir_lowering=False, debug=False,
                   num_devices=NCORES)
    xin_d = nc.dram_tensor("xin", [NPAIR, 2 * CT, 1 + T], F32R,
                           kind="ExternalInput")
    w0_d = nc.dram_tensor("w0", [4 * CT, 3, 128], F32R, kind="ExternalInput")
    wk_d = nc.dram_tensor("wk", [64, 27, 64], F32R, kind="ExternalInput")
    whp_d = nc.dram_tensor("whp", [128, 4], F32R, kind="ExternalInput")
    wha_d = nc.dram_tensor("wha", [128, 2], F32R, kind="ExternalInput")
    wh2_d = nc.dram_tensor("wh2", [4, 2], F32R, kind="ExternalInput")
    bias_d = nc.dram_tensor("bias", [128, 11], F32, kind="ExternalInput")
    bh_d = nc.dram_tensor("bh", [4, 2], F32, kind="ExternalInput")
    out_d = {
        nm: nc.dram_tensor(nm, [BPC, T], F32, kind="ExternalOutput")
        for nm in ("pet", "pck", "aet", "cwd")
    }
    with tile.TileContext(nc) as tc:
        with tc.tile_pool(name="sb", bufs=2) as sb:
            t = sb.tile([BPC, T], F32)
            nc.sync.dma_start(out=t, in_=xin_d.bitcast(F32)[0, 0:BPC, 1:1 + T])
            for nm in ("pet", "pck", "aet", "cwd"):
                nc.sync.dma_start(out=out_d[nm][:], in_=t)
    nc.compile()
    return nc

